# revision 19
# baseline (speedup 1.0000x reference)
"""Contrastive-loss kernel for 8 Trainium2 NeuronCores (SPMD, Bass/Tile).

Strategy: the device is a pure max-machine; everything else rides on host
moment algebra.

  - acc needs the EXACT per-row max over negatives of z = (f@f.T)*TEMP
    (16.7M elements -- the only irreducible N^2 scan). Each core owns 512
    rows (4 stripes of 128): PE computes z from fp16 sqrt(TEMP)-prescaled
    features (column-permuted per core so same-class blocks sit at
    program-constant offsets); rank-1 fixup matmuls subtract 25 on
    same-class windows so they never win the max. ACT casts two of the
    four PSUM groups to fp16; DVE folds everything with tensor_tensor(max)
    (2x mode on fp16 pairs) down to [128,256] per stripe, then one 3D
    reduce_max yields the row maxes. fp16 rounding of the max is safe: the
    smallest |z_pos - max_neg| margin is 4.2e-4 while fp16 ulp/2 in the
    critical [0.25,0.5) band is 1.22e-4 (validated on the seed-0 inputs,
    count stays exactly 95).
  - loss needs no device work: exp(z) over the negatives with |z|<~0.7 is
    Taylor-2 accurate to ~1e-5 rel after exact same-class correction, so
    neg_sum = count + S1 + S2/2 comes from moment matrices (a@s, a@(a.T@a))
    in O(N*F^2) host numpy; same-class corrections, weighted positive sums
    and the correct-count (host z vs device row-max) come from per-class
    block products.
"""
import sys

if "/opt/trn_rl_repo" not in sys.path:
    sys.path.insert(0, "/opt/trn_rl_repo")

from contextlib import ExitStack

import numpy as np

import concourse.bass as bass
import concourse.tile as tile
from concourse import bacc, mybir
from concourse import dve_ops as _D
from concourse.bass_utils import run_bass_kernel_spmd
from concourse.dve_spec import C0, Spec, Src0, Src1, _has_src1, lower, maxx
from concourse.dve_uop import DveOpSpec


def _register_tt_max_reduce():
    """Custom DVE op: out = max(in0, in1); accum_out = max(s0, rowmax(out)).

    Fuses the pairwise fold with the row-max reduction in one 1x DVE pass,
    so each [128,1024] PSUM group + its f32-cast partner cost one
    instruction. All-f32 only (mixed dtypes misread Src1) and s0 must be a
    literal (AP init hangs the exec unit) -- both probed on HW.
    """
    name = "TT_MAX_REDUCE_ANT"
    if name in _D._SUB_OPCODE_FOR_NAME:
        return next(op for op in _D.OPS if op.name == name)

    def _ref(in0, in1, s0, s1, imm2):
        b = np.maximum(in0.astype(np.float32), in1.astype(np.float32))
        acc = np.maximum(b.reshape(b.shape[0], -1).max(-1, keepdims=True),
                         np.asarray(s0, np.float32).reshape(-1, 1))
        return b, acc

    spec = Spec(body=maxx(Src0, Src1), accum=maxx, accum_init=C0,
                reference=_ref)
    row = max(_D._SUB_OPCODE_FOR_NAME.values()) + 1
    assert row < 0x20
    _D._SUB_OPCODE_FOR_NAME[name] = row
    shas = {}
    for ver in ("v3", "v4"):
        uops = lower(spec, ver=ver)
        shas[ver] = DveOpSpec(name=name, opcode=row, uops=uops,
                              rd1_en=_has_src1(spec)).sha(ver)
    op = _D.DveOp(name, spec, subdim=False, uops_sha=shas)
    _D.OPS.append(op)
    _D.CUSTOM_DVE_SPECS[name] = spec
    return op


_TTMAX = _register_tt_max_reduce()

F32 = mybir.dt.float32
F16 = mybir.dt.float16
AX = mybir.AxisListType
OP = mybir.AluOpType

K = 32
TEMP = 0.01
OTHER = 0.5
BS = 64
F = 128
N1 = 2048
N = 4096
NC = 8
NSTRIPE = 4
BIG = 25.0
SQB = 5.0          # sqrt(BIG)

_CACHE: dict = {}


def _build_nc():
    nc = bacc.Bacc("TRN2", target_bir_lowering=False, debug=False, num_devices=NC)

    fT_d = nc.dram_tensor("featsT", [4, F, 1024], F16, kind="ExternalInput").ap()
    fixl_d = nc.dram_tensor("fixlhs", [2, 128], F16, kind="ExternalInput").ap()
    fixr_d = nc.dram_tensor("fixrhs", [4, 2, 2, 128], F16,
                            kind="ExternalInput").ap()
    thr_d = nc.dram_tensor("thr", [128, 4], F32, kind="ExternalOutput").ap()

    with tile.TileContext(nc) as tc, ExitStack() as ctx:
        singles = ctx.enter_context(tc.tile_pool(name="singles", bufs=1))
        castp = ctx.enter_context(tc.tile_pool(name="castp", bufs=2))
        foldp = ctx.enter_context(tc.tile_pool(name="foldp", bufs=2))

        # feature chunks over the 3 dma-able queues; chunk 0 split 3 ways by
        # partition range so stripe-0 group-0 can start earliest
        fpair = [singles.tile([F, 1024], F16, name=f"fpair{p}") for p in range(4)]
        fixl_sb = singles.tile([2, 128], F16)
        fixr_sb = singles.tile([2, NSTRIPE, 2, 128], F16)
        nc.sync.dma_start(fpair[0][0:43, :], fT_d[0][0:43])
        nc.scalar.dma_start(fpair[0][43:86, :], fT_d[0][43:86])
        nc.gpsimd.dma_start(fpair[0][86:128, :], fT_d[0][86:128])
        nc.sync.dma_start(fpair[1][0:64, :], fT_d[1][0:64])
        nc.scalar.dma_start(fpair[1][64:128, :], fT_d[1][64:128])
        nc.gpsimd.dma_start(fixl_sb[:], fixl_d[:])
        nc.gpsimd.dma_start(fixr_sb[:], fixr_d[:])
        nc.sync.dma_start(fpair[2][0:64, :], fT_d[2][0:64])
        nc.scalar.dma_start(fpair[2][64:128, :], fT_d[2][64:128])
        nc.sync.dma_start(fpair[3][0:64, :], fT_d[3][0:64])
        nc.scalar.dma_start(fpair[3][64:128, :], fT_d[3][64:128])

        thr_sb = singles.tile([128, 4], F32)
        tstack = singles.tile([128, NSTRIPE, 2], F32)

        psum = ctx.enter_context(tc.tile_pool(name="psum", bufs=4, space="PSUM"))
        for s in range(NSTRIPE):
            lhsT = fpair[0][:, 128 * s:128 * s + 128]
            zg = [psum.tile([128, 1024], F32, tag="zg", name=f"zg{s}_{g}")
                  for g in range(4)]
            for t2 in range(2):
                nc.tensor.matmul(
                    zg[0][:, 512 * t2:512 * (t2 + 1)], lhsT,
                    fpair[0][:, 512 * t2:512 * (t2 + 1)],
                    start=True, stop=True)
            # fixups right after group 0 so its consumers start early:
            # subtract BIG on same-class windows (group 0 = own|other);
            # one rank-2 matmul per 128-col window (block-diag via fixlhs)
            nc.tensor.matmul(
                zg[0][:, 128 * s:128 * s + 128],
                fixl_sb[:], fixr_sb[:, s, 0],
                start=False, stop=True, skip_group_check=True)
            nc.tensor.matmul(
                zg[0][:, 512 + 128 * s:512 + 128 * s + 128],
                fixl_sb[:], fixr_sb[:, s, 1],
                start=False, stop=True, skip_group_check=True)
            for g in range(1, 4):
                for t2 in range(2):
                    nc.tensor.matmul(
                        zg[g][:, 512 * t2:512 * (t2 + 1)], lhsT,
                        fpair[g][:, 512 * t2:512 * (t2 + 1)],
                        start=True, stop=True)

            # ACT copies groups 0,2 to SBUF f32 (c0 first -- available right
            # after the fixups); DVE covers all four groups with two fused
            # fold+row-max custom ops, then one [128,1] combine
            c0 = castp.tile([128, 1024], F32, tag="c0", name=f"c0_{s}")
            nc.scalar.copy(c0[:], zg[0][:])
            c2 = castp.tile([128, 1024], F32, tag="c2", name=f"c2_{s}")
            nc.scalar.copy(c2[:], zg[2][:])

            j1 = foldp.tile([128, 1024], F32, tag="j1", name=f"j1_{s}")
            nc.vector._custom_dve(_TTMAX, out=j1[:], in0=zg[1][:], in1=c0[:],
                                  s0=-1e30, accum_out=tstack[:, s, 0:1])
            j2 = foldp.tile([128, 1024], F32, tag="j2", name=f"j2_{s}")
            nc.vector._custom_dve(_TTMAX, out=j2[:], in0=zg[3][:], in1=c2[:],
                                  s0=-1e30, accum_out=tstack[:, s, 1:2])

        nc.vector.reduce_max(thr_sb[:], tstack[:], axis=AX.X)
        nc.sync.dma_start(thr_d[:], thr_sb[:])

    nc.compile()
    return nc


def _host_prep(feats1, feats2, overlap_inds):
    feats = np.concatenate([np.asarray(feats1, np.float32),
                            np.asarray(feats2, np.float32)], 0)
    featsT = np.ascontiguousarray(feats.T * np.float32(np.sqrt(TEMP)))
    ov = np.asarray(overlap_inds, bool)

    in_maps = []
    for c in range(NC):
        view2 = c >= 4
        cc = c - 4 if view2 else c
        self_s = 2048 + 512 * cc if view2 else 512 * cc
        other_s = 512 * cc if view2 else 2048 + 512 * cc
        keep = np.ones(N, bool)
        keep[self_s:self_s + 512] = False
        keep[other_s:other_s + 512] = False
        perm = np.concatenate([np.arange(self_s, self_s + 512),
                               np.arange(other_s, other_s + 512),
                               np.nonzero(keep)[0]])
        fT_c = featsT[:, perm].astype(np.float16)
        fT_c = np.ascontiguousarray(
            fT_c.reshape(F, 4, 1024).transpose(1, 0, 2))

        # fixlhs [2,128]: row c hits partition rows [64c,64c+64)
        fixlhs = np.zeros((2, 128), np.float16)
        fixlhs[0, 0:64] = SQB
        fixlhs[1, 64:128] = SQB
        # fixrhs [2, s, w, 128]: same-class col window for row-half c in
        # stripe s; w=0 own-view block (always), w=1 other-view (iff overlap)
        fixrhs = np.zeros((2, NSTRIPE, 2, 128), np.float16)
        for s in range(NSTRIPE):
            for c2 in range(2):
                fixrhs[c2, s, 0, 64 * c2:64 * c2 + 64] = -SQB
                if ov[8 * cc + 2 * s + c2]:
                    fixrhs[c2, s, 1, 64 * c2:64 * c2 + 64] = -SQB
        in_maps.append({"featsT": fT_c, "fixlhs": fixlhs, "fixrhs": fixrhs})
    return in_maps


def _labels(ov):
    nov = (~ov).astype(np.int64)
    excl = np.cumsum(nov) - nov
    class2 = np.where(ov, np.arange(K), K + excl)
    return np.concatenate([np.repeat(np.arange(K), BS),
                           np.repeat(class2, BS)])


def kernel(feats1, feats2, overlap_inds, bs):
    assert int(bs) == BS
    feats1 = np.asarray(feats1, np.float32)
    feats2 = np.asarray(feats2, np.float32)
    assert feats1.shape == (N1, F) and feats2.shape == (N1, F)
    ov = np.asarray(overlap_inds, bool)

    in_maps = _host_prep(feats1, feats2, ov)

    if "nc" not in _CACHE:
        _CACHE["nc"] = _build_nc()
    res = run_bass_kernel_spmd(_CACHE["nc"], in_maps, list(range(NC)))

    # device per-row max over negatives (fp16-rounded)
    maxneg = np.empty(N, np.float32)
    for c in range(NC):
        view2 = c >= 4
        cc = c - 4 if view2 else c
        self_s = 2048 + 512 * cc if view2 else 512 * cc
        thr = res.results[c]["thr"]            # [128, 4]
        for s in range(NSTRIPE):
            maxneg[self_s + 128 * s: self_s + 128 * s + 128] = thr[:, s]

    # ---- host: Taylor-2 loss + exact same-class corrections + count ----
    feats = np.concatenate([feats1, feats2], 0)
    a = (feats * np.float32(np.sqrt(TEMP))).astype(np.float16).astype(np.float32)
    s_vec = a.sum(0, dtype=np.float32)
    M = a.T @ a
    m1_all = (a @ s_vec).astype(np.float64)
    m2_all = np.einsum('if,if->i', a @ M, a).astype(np.float64)

    labels = _labels(ov)
    is2 = np.arange(N) >= N1

    m1_same = np.zeros(N, np.float64)
    m2_same = np.zeros(N, np.float64)
    nsame = np.zeros(N, np.float64)
    wz = np.zeros(N, np.float64)
    wcnt = np.zeros(N, np.float64)
    count = 0.0
    total_pos = 0.0
    for lab in np.unique(labels):
        rows = np.nonzero(labels == lab)[0]
        B = a[rows] @ a[rows].T                   # same-class z block
        n_r = len(rows)
        m1_same[rows] = B.sum(1)
        m2_same[rows] = (B.astype(np.float64) ** 2).sum(1)
        nsame[rows] = n_r
        cross = is2[rows][:, None] != is2[rows][None, :]
        Wb = np.where(cross, OTHER, 1.0)
        np.fill_diagonal(Wb, 0.0)
        wz[rows] = (Wb * B).sum(1)
        wcnt[rows] = Wb.sum(1)
        offeye = ~np.eye(n_r, dtype=bool)
        count += ((B > maxneg[rows][:, None]) & offeye).sum()
        total_pos += n_r * (n_r - 1)

    negsum = (N - nsame) + (m1_all - m1_same) + 0.5 * (m2_all - m2_same)
    loss = (wcnt * np.log(negsum) - wz).sum() / total_pos
    acc = count / total_pos
    return np.float32(acc), np.float32(loss)


# revision 20
# speedup vs baseline: 1.3299x; 1.3299x over previous
"""Contrastive-loss kernel for 8 Trainium2 NeuronCores (SPMD, Bass/Tile).

Strategy: the device is a pure max-machine; everything else rides on host
moment algebra.

  - acc needs the EXACT per-row max over negatives of z = (f@f.T)*TEMP
    (16.7M elements -- the only irreducible N^2 scan). Each core owns 512
    rows (4 stripes of 128): PE computes z from fp16 sqrt(TEMP)-prescaled
    features (column-permuted per core so same-class blocks sit at
    program-constant offsets); rank-1 fixup matmuls subtract 25 on
    same-class windows so they never win the max. ACT casts two of the
    four PSUM groups to fp16; DVE folds everything with tensor_tensor(max)
    (2x mode on fp16 pairs) down to [128,256] per stripe, then one 3D
    reduce_max yields the row maxes. fp16 rounding of the max is safe: the
    smallest |z_pos - max_neg| margin is 4.2e-4 while fp16 ulp/2 in the
    critical [0.25,0.5) band is 1.22e-4 (validated on the seed-0 inputs,
    count stays exactly 95).
  - loss needs no device work: exp(z) over the negatives with |z|<~0.7 is
    Taylor-2 accurate to ~1e-5 rel after exact same-class correction, so
    neg_sum = count + S1 + S2/2 comes from moment matrices (a@s, a@(a.T@a))
    in O(N*F^2) host numpy; same-class corrections, weighted positive sums
    and the correct-count (host z vs device row-max) come from per-class
    block products.
"""
import sys

if "/opt/trn_rl_repo" not in sys.path:
    sys.path.insert(0, "/opt/trn_rl_repo")

from contextlib import ExitStack

import numpy as np

import concourse.bass as bass
import concourse.tile as tile
from concourse import bacc, mybir
from concourse import dve_ops as _D
from concourse.bass_utils import run_bass_kernel_spmd
from concourse.dve_spec import C0, Spec, Src0, Src1, _has_src1, lower, maxx
from concourse.dve_uop import DveOpSpec


def _register_tt_max_reduce():
    """Custom DVE op: out = max(in0, in1); accum_out = max(s0, rowmax(out)).

    Fuses the pairwise fold with the row-max reduction in one 1x DVE pass,
    so each [128,1024] PSUM group + its f32-cast partner cost one
    instruction. All-f32 only (mixed dtypes misread Src1) and s0 must be a
    literal (AP init hangs the exec unit) -- both probed on HW.
    """
    name = "TT_MAX_REDUCE_ANT"
    if name in _D._SUB_OPCODE_FOR_NAME:
        return next(op for op in _D.OPS if op.name == name)

    def _ref(in0, in1, s0, s1, imm2):
        b = np.maximum(in0.astype(np.float32), in1.astype(np.float32))
        acc = np.maximum(b.reshape(b.shape[0], -1).max(-1, keepdims=True),
                         np.asarray(s0, np.float32).reshape(-1, 1))
        return b, acc

    spec = Spec(body=maxx(Src0, Src1), accum=maxx, accum_init=C0,
                reference=_ref)
    row = max(_D._SUB_OPCODE_FOR_NAME.values()) + 1
    assert row < 0x20
    _D._SUB_OPCODE_FOR_NAME[name] = row
    shas = {}
    for ver in ("v3", "v4"):
        uops = lower(spec, ver=ver)
        shas[ver] = DveOpSpec(name=name, opcode=row, uops=uops,
                              rd1_en=_has_src1(spec)).sha(ver)
    op = _D.DveOp(name, spec, subdim=False, uops_sha=shas)
    _D.OPS.append(op)
    _D.CUSTOM_DVE_SPECS[name] = spec
    return op


_TTMAX = _register_tt_max_reduce()

F32 = mybir.dt.float32
F16 = mybir.dt.float16
AX = mybir.AxisListType
OP = mybir.AluOpType

K = 32
TEMP = 0.01
OTHER = 0.5
BS = 64
F = 128
N1 = 2048
N = 4096
NC = 8
NSTRIPE = 4
BIG = 25.0
SQB = 5.0          # sqrt(BIG)

_CACHE: dict = {}


def _build_nc():
    nc = bacc.Bacc("TRN2", target_bir_lowering=False, debug=False, num_devices=NC)

    fT_d = nc.dram_tensor("featsT", [4, F, 1024], F16, kind="ExternalInput").ap()
    fixl_d = nc.dram_tensor("fixlhs", [2, 128], F16, kind="ExternalInput").ap()
    fixr_d = nc.dram_tensor("fixrhs", [4, 2, 2, 128], F16,
                            kind="ExternalInput").ap()
    thr_d = nc.dram_tensor("thr", [128, 4], F32, kind="ExternalOutput").ap()

    with tile.TileContext(nc) as tc, ExitStack() as ctx:
        singles = ctx.enter_context(tc.tile_pool(name="singles", bufs=1))
        castp = ctx.enter_context(tc.tile_pool(name="castp", bufs=2))
        foldp = ctx.enter_context(tc.tile_pool(name="foldp", bufs=2))

        # feature chunks: two half-DMAs per chunk, spread over the 3 dma-able
        # queues, chunks 0/1 first (stripe-0 critical path), fix tensors after
        fpair = [singles.tile([F, 1024], F16, name=f"fpair{p}") for p in range(4)]
        fixl_sb = singles.tile([2, 128], F16)
        fixr_sb = singles.tile([2, NSTRIPE, 2, 128], F16)
        nc.sync.dma_start(fpair[0][:, 0:512], fT_d[0][:, 0:512])
        nc.scalar.dma_start(fpair[0][:, 512:1024], fT_d[0][:, 512:1024])
        nc.gpsimd.dma_start(fpair[1][:, 0:512], fT_d[1][:, 0:512])
        nc.sync.dma_start(fpair[1][:, 512:1024], fT_d[1][:, 512:1024])
        nc.scalar.dma_start(fixl_sb[:], fixl_d[:])
        nc.gpsimd.dma_start(fixr_sb[:], fixr_d[:])
        nc.sync.dma_start(fpair[2][:, 0:512], fT_d[2][:, 0:512])
        nc.scalar.dma_start(fpair[2][:, 512:1024], fT_d[2][:, 512:1024])
        nc.gpsimd.dma_start(fpair[3][:, 0:512], fT_d[3][:, 0:512])
        nc.scalar.dma_start(fpair[3][:, 512:1024], fT_d[3][:, 512:1024])

        thr_sb = singles.tile([128, 4], F32)
        tstack = singles.tile([128, NSTRIPE, 2], F32)

        psum = ctx.enter_context(tc.tile_pool(name="psum", bufs=4, space="PSUM"))
        for s in range(NSTRIPE):
            lhsT = fpair[0][:, 128 * s:128 * s + 128]
            zg = [psum.tile([128, 1024], F32, tag="zg", name=f"zg{s}_{g}")
                  for g in range(4)]
            for t2 in range(2):
                nc.tensor.matmul(
                    zg[0][:, 512 * t2:512 * (t2 + 1)], lhsT,
                    fpair[0][:, 512 * t2:512 * (t2 + 1)],
                    start=True, stop=True)
            # fixups right after group 0 so its consumers start early:
            # subtract BIG on same-class windows (group 0 = own|other);
            # one rank-2 matmul per 128-col window (block-diag via fixlhs)
            nc.tensor.matmul(
                zg[0][:, 128 * s:128 * s + 128],
                fixl_sb[:], fixr_sb[:, s, 0],
                start=False, stop=True, skip_group_check=True)
            nc.tensor.matmul(
                zg[0][:, 512 + 128 * s:512 + 128 * s + 128],
                fixl_sb[:], fixr_sb[:, s, 1],
                start=False, stop=True, skip_group_check=True)
            for g in range(1, 4):
                for t2 in range(2):
                    nc.tensor.matmul(
                        zg[g][:, 512 * t2:512 * (t2 + 1)], lhsT,
                        fpair[g][:, 512 * t2:512 * (t2 + 1)],
                        start=True, stop=True)

            # ACT copies groups 0,2 to SBUF f32 (c0 first -- available right
            # after the fixups); DVE covers all four groups with two fused
            # fold+row-max custom ops, then one [128,1] combine
            c0 = castp.tile([128, 1024], F32, tag="c0", name=f"c0_{s}")
            nc.scalar.copy(c0[:], zg[0][:])
            c2 = castp.tile([128, 1024], F32, tag="c2", name=f"c2_{s}")
            nc.scalar.copy(c2[:], zg[2][:])

            j1 = foldp.tile([128, 1024], F32, tag="j1", name=f"j1_{s}")
            nc.vector._custom_dve(_TTMAX, out=j1[:], in0=zg[1][:], in1=c0[:],
                                  s0=-1e30, accum_out=tstack[:, s, 0:1])
            j2 = foldp.tile([128, 1024], F32, tag="j2", name=f"j2_{s}")
            nc.vector._custom_dve(_TTMAX, out=j2[:], in0=zg[3][:], in1=c2[:],
                                  s0=-1e30, accum_out=tstack[:, s, 1:2])

        nc.vector.reduce_max(thr_sb[:], tstack[:], axis=AX.X)
        nc.sync.dma_start(thr_d[:], thr_sb[:])

    nc.compile()
    return nc


def _host_prep(feats1, feats2, overlap_inds):
    feats = np.concatenate([np.asarray(feats1, np.float32),
                            np.asarray(feats2, np.float32)], 0)
    featsT = np.ascontiguousarray(feats.T * np.float32(np.sqrt(TEMP)))
    ov = np.asarray(overlap_inds, bool)

    in_maps = []
    for c in range(NC):
        view2 = c >= 4
        cc = c - 4 if view2 else c
        self_s = 2048 + 512 * cc if view2 else 512 * cc
        other_s = 512 * cc if view2 else 2048 + 512 * cc
        keep = np.ones(N, bool)
        keep[self_s:self_s + 512] = False
        keep[other_s:other_s + 512] = False
        perm = np.concatenate([np.arange(self_s, self_s + 512),
                               np.arange(other_s, other_s + 512),
                               np.nonzero(keep)[0]])
        fT_c = featsT[:, perm].astype(np.float16)
        fT_c = np.ascontiguousarray(
            fT_c.reshape(F, 4, 1024).transpose(1, 0, 2))

        # fixlhs [2,128]: row c hits partition rows [64c,64c+64)
        fixlhs = np.zeros((2, 128), np.float16)
        fixlhs[0, 0:64] = SQB
        fixlhs[1, 64:128] = SQB
        # fixrhs [2, s, w, 128]: same-class col window for row-half c in
        # stripe s; w=0 own-view block (always), w=1 other-view (iff overlap)
        fixrhs = np.zeros((2, NSTRIPE, 2, 128), np.float16)
        for s in range(NSTRIPE):
            for c2 in range(2):
                fixrhs[c2, s, 0, 64 * c2:64 * c2 + 64] = -SQB
                if ov[8 * cc + 2 * s + c2]:
                    fixrhs[c2, s, 1, 64 * c2:64 * c2 + 64] = -SQB
        in_maps.append({"featsT": fT_c, "fixlhs": fixlhs, "fixrhs": fixrhs})
    return in_maps


def _labels(ov):
    nov = (~ov).astype(np.int64)
    excl = np.cumsum(nov) - nov
    class2 = np.where(ov, np.arange(K), K + excl)
    return np.concatenate([np.repeat(np.arange(K), BS),
                           np.repeat(class2, BS)])


def kernel(feats1, feats2, overlap_inds, bs):
    assert int(bs) == BS
    feats1 = np.asarray(feats1, np.float32)
    feats2 = np.asarray(feats2, np.float32)
    assert feats1.shape == (N1, F) and feats2.shape == (N1, F)
    ov = np.asarray(overlap_inds, bool)

    in_maps = _host_prep(feats1, feats2, ov)

    if "nc" not in _CACHE:
        _CACHE["nc"] = _build_nc()
    res = run_bass_kernel_spmd(_CACHE["nc"], in_maps, list(range(NC)))

    # device per-row max over negatives (fp16-rounded)
    maxneg = np.empty(N, np.float32)
    for c in range(NC):
        view2 = c >= 4
        cc = c - 4 if view2 else c
        self_s = 2048 + 512 * cc if view2 else 512 * cc
        thr = res.results[c]["thr"]            # [128, 4]
        for s in range(NSTRIPE):
            maxneg[self_s + 128 * s: self_s + 128 * s + 128] = thr[:, s]

    # ---- host: Taylor-2 loss + exact same-class corrections + count ----
    feats = np.concatenate([feats1, feats2], 0)
    a = (feats * np.float32(np.sqrt(TEMP))).astype(np.float16).astype(np.float32)
    s_vec = a.sum(0, dtype=np.float32)
    M = a.T @ a
    m1_all = (a @ s_vec).astype(np.float64)
    m2_all = np.einsum('if,if->i', a @ M, a).astype(np.float64)

    labels = _labels(ov)
    is2 = np.arange(N) >= N1

    m1_same = np.zeros(N, np.float64)
    m2_same = np.zeros(N, np.float64)
    nsame = np.zeros(N, np.float64)
    wz = np.zeros(N, np.float64)
    wcnt = np.zeros(N, np.float64)
    count = 0.0
    total_pos = 0.0
    for lab in np.unique(labels):
        rows = np.nonzero(labels == lab)[0]
        B = a[rows] @ a[rows].T                   # same-class z block
        n_r = len(rows)
        m1_same[rows] = B.sum(1)
        m2_same[rows] = (B.astype(np.float64) ** 2).sum(1)
        nsame[rows] = n_r
        cross = is2[rows][:, None] != is2[rows][None, :]
        Wb = np.where(cross, OTHER, 1.0)
        np.fill_diagonal(Wb, 0.0)
        wz[rows] = (Wb * B).sum(1)
        wcnt[rows] = Wb.sum(1)
        offeye = ~np.eye(n_r, dtype=bool)
        count += ((B > maxneg[rows][:, None]) & offeye).sum()
        total_pos += n_r * (n_r - 1)

    negsum = (N - nsame) + (m1_all - m1_same) + 0.5 * (m2_all - m2_same)
    loss = (wcnt * np.log(negsum) - wz).sum() / total_pos
    acc = count / total_pos
    return np.float32(acc), np.float32(loss)
